# revision 22
# baseline (speedup 1.0000x reference)
"""Trainium2 Bass kernel for MQA sliding-window causal self-attention.

Sharding: 8 cores = DP(batch=2) x TP(head-groups=4). Each core computes 4 of
16 query heads for one batch element, with the shared KV head replicated.
Host: prepares pre-tiled/concatenated weight+input layouts (so every DMA is
one fused contiguous-ish transfer), gathers and sums the 4 tensor-parallel
partial outputs per batch element.

Device pipeline per core:
  phase 1 (4 groups of 4 s-blocks, stage-major software pipeline):
    A: fused QKV+gate projections (fp32r matmuls) -> ACT copy to SBUF
    B1/G: rope (C/S' form) on DVE+GPSIMD
    B2: rmsnorm via r*pow(mean(r^2)+eps, -0.5) on DVE
    V: gate sigma + v_eff = sigma*2ve + v
    C: PE transposes -> qT head-pair tiles, duplicated kT (one DMA per group)
  phase 2 (two head-pair passes; per key-block j):
    scoresT strip (k_j stationary, N<=512 chunks) -> exp on ACT (f32r out)
    -> edge masks -> gather-mm2 (consecutive accumulating matmuls
    attn.T @ [v|1] into per-qblock PSUM accumulator, own bank)
    -> epilogue delayed 1 iteration: reciprocal of denominator row,
    PE ones-broadcast, divide; odd head staged in quad tiles, one DMA per 4
    qblocks; per-s-block output projection trails in pass B.
"""
import numpy as np
from contextlib import ExitStack

import concourse.bass as bass
import concourse.tile as tile
import concourse.mybir as mybir
from concourse import bacc
from concourse.bass_utils import run_bass_kernel_spmd
from concourse.masks import make_identity

F32 = mybir.dt.float32
F32R = mybir.dt.float32r
AF = mybir.ActivationFunctionType
ALU = mybir.AluOpType

B, S, E, H, KV, D = 2, 2048, 1024, 16, 1, 64
HALF = D // 2
GATE_CH = 32
WIN = 1024
NCORES = 8
TP = 4
HPC = H // TP            # heads per core = 4
HD = HPC * D             # per-core q width = 256
SB = S // 128            # 16 s-blocks
JB = S // 128            # 16 key blocks
WB = WIN // 128          # 8 window blocks
QKW = HD + 2 * D         # 384
RW = HD + D              # 320 roped width (4 q heads + k)
GW = 4                   # s-blocks per phase-1 group
NG = SB // GW            # 4 groups

TRACE = False
LAST_RESULT = [None]
_NC_CACHE = [None]


def _build(phases=(1, 2, 3)):
    nc = bacc.Bacc()

    xTg = nc.dram_tensor("xTg", [NG, 8, 128, GW * 128], F32R, kind="ExternalInput")
    wq_t = nc.dram_tensor("wq_t", [8, 128, QKW], F32R, kind="ExternalInput")
    wg = nc.dram_tensor("wg", [GATE_CH, 2], F32R, kind="ExternalInput")
    wo = nc.dram_tensor("wo", [HD, E], F32R, kind="ExternalInput")
    tgve = nc.dram_tensor("tgve", [SB, 128, 192], F32, kind="ExternalInput")
    maskd = nc.dram_tensor("maskd", [128, 2 * 128], F32, kind="ExternalInput")
    maskf = nc.dram_tensor("maskf", [128, 2 * 128], F32, kind="ExternalInput")
    y = nc.dram_tensor("y", [SB, 128, E], F32, kind="ExternalOutput")

    with tile.TileContext(nc) as tc, ExitStack() as top:
        const = top.enter_context(tc.tile_pool(name="const", bufs=1))
        persist = top.enter_context(tc.tile_pool(name="persist", bufs=1))

        # ---- constants ----
        ident32 = const.tile([128, 128], F32)
        make_identity(nc, ident32)
        ident = const.tile([128, 128], F32R)
        nc.vector.tensor_copy(ident, ident32)
        zero_b = const.tile([128, 1], F32)
        nc.vector.memset(zero_b, 0.0)
        eps_b = const.tile([128, 1], F32)
        nc.vector.memset(eps_b, float(np.finfo(np.float32).eps))
        ones32 = const.tile([128, 64], F32)
        nc.vector.memset(ones32, 1.0)
        ones_r = const.tile([1, 64], F32R)
        nc.vector.tensor_copy(ones_r, ones32[0:1, :])
        maskd_sb = const.tile([128, 2, 128], F32)
        maskf_sb = const.tile([128, 2, 128], F32)
        wg_sb = const.tile([GATE_CH, 2], F32R)
        nc.sync.dma_start(wg_sb, wg[:, :])

        # ---- persistent activations ----
        qT = [persist.tile([128, S], F32R, tag=f"qT{i}", name=f"qT{i}") for i in range(2)]
        kT2 = persist.tile([128, S], F32R)
        vext = persist.tile([128, JB, D + 1], F32R)
        aoT = [persist.tile([128, S], F32R, tag=f"aoT{i}", name=f"aoT{i}") for i in range(2)]
        nc.vector.tensor_copy(vext[:, :, D], ones32[:, 0:JB])

        wo_sb = [persist.tile([128, E], F32R, tag=f"wo{i}", name=f"wo{i}") for i in range(2)]

        # =============== phase 1: projections + rope + rmsnorm ===============
        with ExitStack() as p1:
            xpool = p1.enter_context(tc.tile_pool(name="xg", bufs=2))
            wpool = p1.enter_context(tc.tile_pool(name="wqkv", bufs=1))
            trig = p1.enter_context(tc.tile_pool(name="trig", bufs=2))
            work = p1.enter_context(tc.tile_pool(name="work", bufs=8))
            qkvp = p1.enter_context(tc.tile_pool(name="qkvsb", bufs=12))
            qkv_psp = p1.enter_context(tc.tile_pool(name="qkv_ps", bufs=4, space="PSUM"))
            tr_psp = p1.enter_context(tc.tile_pool(name="tr_ps", bufs=3, space="PSUM"))
            g_psp = p1.enter_context(tc.tile_pool(name="g_ps", bufs=1, space="PSUM"))

            wq_all = wpool.tile([128, 8, QKW], F32R)
            nc.sync.dma_start(
                wq_all,
                bass.AP(tensor=wq_t, offset=0,
                        ap=[[QKW, 128], [128 * QKW, 8], [1, QKW]]))

            xg = {}
            tgv = {}

            def load_group(g):
                t = xpool.tile([128, 8, GW * 128], F32R, tag="xg", name="xg_t", bufs=3)
                nc.sync.dma_start(
                    t, bass.AP(tensor=xTg, offset=g * 8 * 128 * GW * 128,
                               ap=[[GW * 128, 128], [128 * GW * 128, 8],
                                   [1, GW * 128]]))
                xg[g] = t
                tv = trig.tile([128, GW, 192], F32, tag="tgve", name="tgve_t",
                               bufs=3)
                nc.sync.dma_start(
                    tv, bass.AP(tensor=tgve, offset=g * GW * 128 * 192,
                                ap=[[192, 128], [128 * 192, GW], [1, 192]]))
                tgv[g] = tv

            gate_ps = g_psp.tile([128, SB, 2], F32)
            st = {}          # per-group stage state

            def stage_a(g):
                qkvg = qkvp.tile([128, GW, QKW], F32, tag="qkvg", name="qkvg",
                                 bufs=3)
                for li in range(GW):
                    sb = g * GW + li
                    lcol = slice(li * 128, (li + 1) * 128)
                    qkv_ps = qkv_psp.tile([128, QKW], F32, tag="qkv", name="qkv_ps")
                    for k in range(8):
                        nc.tensor.matmul(qkv_ps, xg[g][:, k, lcol],
                                         wq_all[:, k, :],
                                         start=(k == 0), stop=(k == 7),
                                         skip_group_check=True)
                    nc.tensor.matmul(gate_ps[:, sb, :], xg[g][0:GATE_CH, 0, lcol],
                                     wg_sb[:], start=True, stop=True)
                    nc.scalar.copy(qkvg[:, li, :], qkv_ps)
                st[g] = dict(qkvg=qkvg, tgv=tgv[g])

            def tgbc(tv, lo, width):
                # [128, li, (5 heads bcast), width] slice of the tgve group tile
                return bass.AP(tensor=tv.tensor, offset=tv.offset + lo,
                               ap=[list(tv.ap[0]), [192, GW], [0, 5], [1, width]])

            def stage_b1(g):
                s_ = st[g]
                qkvg, tv = s_["qkvg"], s_["tgv"]
                tm1 = work.tile([128, GW, RW], F32, tag="tm1", name="tm1", bufs=3)
                tm2 = work.tile([128, GW, RW], F32, tag="tm2", name="tm2", bufs=2)
                qk5 = qkvg[:, :, 0:RW].rearrange("p l (h d) -> p l h d", h=5)
                tm1v = tm1[:].rearrange("p l (h d) -> p l h d", h=5)
                tm2v = tm2[:].rearrange("p l (h d) -> p l h d", h=5)
                nc.vector.tensor_mul(tm1v, qk5, tgbc(tv, 0, D))
                nc.gpsimd.tensor_mul(tm2v[:, :, :, 0:HALF], qk5[:, :, :, HALF:D],
                                     tgbc(tv, D, HALF))
                nc.gpsimd.tensor_mul(tm2v[:, :, :, HALF:D], qk5[:, :, :, 0:HALF],
                                     tgbc(tv, D + HALF, HALF))
                st[g].update(tm1=tm1, tm2=tm2)

            def stage_g(g):
                s_ = st[g]
                qk_r = work.tile([128, GW, RW], F32, tag="qkr", name="qk_r",
                                 bufs=2)
                nc.gpsimd.tensor_add(qk_r, s_["tm1"], s_["tm2"])
                # square written over tm1 (dead after the add)
                nc.gpsimd.tensor_mul(s_["tm1"], qk_r, qk_r)
                st[g].update(qk_r=qk_r, sq=s_["tm1"])

            def stage_b2(g):
                s_ = st[g]
                qk_r, sq = s_["qk_r"], s_["sq"]
                ms = work.tile([128, GW * 5], F32, tag="ms", name="ms", bufs=2)
                nc.vector.reduce_sum(
                    ms, sq[:].rearrange("p l (h d) -> p (l h) d", h=5),
                    axis=mybir.AxisListType.X)
                lms = work.tile([128, GW * 5], F32, tag="lms", name="lms", bufs=2)
                nc.scalar.activation(lms, ms, AF.Ln, bias=eps_b, scale=1.0 / D)
                rstd = work.tile([128, GW * 5], F32, tag="rstd", name="rstd",
                                 bufs=2)
                nc.scalar.activation(rstd, lms, AF.Exp, bias=zero_b, scale=-0.5)
                qk_n = work.tile([128, GW, RW], F32R, tag="qkn", name="qk_n",
                                 bufs=2)
                rbc = bass.AP(tensor=rstd.tensor, offset=rstd.offset,
                              ap=[list(rstd.ap[0]), [1, GW * 5], [0, D]])
                nc.vector.tensor_mul(
                    qk_n[:].rearrange("p l (h d) -> p (l h) d", h=5),
                    qk_r[:].rearrange("p l (h d) -> p (l h) d", h=5), rbc)
                st[g]["qk_n"] = qk_n

            def stage_v(g):
                s_ = st[g]
                qkvg, tv = s_["qkvg"], s_["tgv"]
                ge = work.tile([128, GW], F32, tag="ge", name="ge", bufs=2)
                nc.scalar.activation(ge, gate_ps[:, g * GW:(g + 1) * GW, 0:1],
                                     AF.Exp, bias=zero_b, scale=-1.0)
                gd = work.tile([128, GW], F32, tag="gd", name="gd", bufs=2)
                nc.vector.tensor_scalar_add(gd, ge, 1.0)
                sig = work.tile([128, GW], F32, tag="sig", name="sig", bufs=2)
                nc.vector.reciprocal(sig, gd)
                for li in range(GW):
                    sb = g * GW + li
                    nc.vector.scalar_tensor_tensor(
                        out=vext[:, sb, 0:D], in0=tv[:, li, 128:192],
                        scalar=sig[:, li:li + 1], in1=qkvg[:, li, RW:QKW],
                        op0=ALU.mult, op1=ALU.add)

            def stage_c(g):
                qk_n = st.pop(g)["qk_n"]
                for li in range(GW):
                    sb = g * GW + li
                    scol = slice(sb * 128, (sb + 1) * 128)
                    for i in range(2):
                        tr_ps = tr_psp.tile([128, 128], F32R, tag="tr",
                                            name="tr_ps")
                        nc.tensor.transpose(tr_ps,
                                            qk_n[:, li, i * 128:(i + 1) * 128],
                                            ident[:])
                        nc.scalar.copy(qT[i][:, scol], tr_ps)
                    trk_ps = tr_psp.tile([128, 128], F32R, tag="tr",
                                         name="trk_ps")
                    nc.tensor.transpose(trk_ps[0:D, :],
                                        qk_n[:, li, HPC * D:RW], ident[:])
                    nc.vector.tensor_copy(kT2[0:D, scol], trk_ps[0:D, :])
                gcol = slice(g * GW * 128, (g + 1) * GW * 128)
                nc.sync.dma_start(kT2[D:128, gcol], kT2[0:D, gcol])

            load_group(0)
            load_group(1)
            for gi in range(NG + 3):
                if gi + 2 < NG:
                    load_group(gi + 2)
                if gi < NG:
                    stage_a(gi)
                if 0 <= gi - 1 < NG:
                    stage_b1(gi - 1)
                    stage_g(gi - 1)
                if 0 <= gi - 2 < NG:
                    stage_b2(gi - 2)
                    stage_v(gi - 2)
                if 0 <= gi - 3 < NG:
                    stage_c(gi - 3)

        # =============== phase 2: attention + interleaved output proj ========
        if 2 in phases:
          with ExitStack() as p2:
            for i in range(2):
                nc.sync.dma_start(wo_sb[i], wo[i * 128:(i + 1) * 128, :])
            nc.sync.dma_start(maskd_sb, maskd[:, :].rearrange("p (h x) -> p h x", h=2))
            nc.sync.dma_start(maskf_sb, maskf[:, :].rearrange("p (h x) -> p h x", h=2))
            strip_psp = p2.enter_context(tc.tile_pool(name="strip", bufs=2, space="PSUM"))
            acc_psp = p2.enter_context(tc.tile_pool(name="acc", bufs=2, space="PSUM"))
            bc_psp = p2.enter_context(tc.tile_pool(name="bc", bufs=1, space="PSUM"))
            y_psp = p2.enter_context(tc.tile_pool(name="y_ps", bufs=1, space="PSUM"))
            expp = p2.enter_context(tc.tile_pool(name="expp", bufs=10))
            ep = p2.enter_context(tc.tile_pool(name="ep", bufs=3))
            yp = p2.enter_context(tc.tile_pool(name="yp", bufs=2))

            scale = float(D) ** -0.5

            for ps in range(2):
                qTp, aoTp = qT[ps], aoT[ps]
                exps = {}
                eps_st = {}
                quad = {}
                mm3_pend = []

                def mm1_exp_masks(j):
                    nq = min(j + WB + 1, SB) - j
                    exp_t = expp.tile([128, 2, (WB + 1) * 128], F32R, tag="exp",
                                      name="exp_t")
                    exps[j] = exp_t
                    jcol = slice(j * 128, (j + 1) * 128)
                    off = 0
                    while off < nq:
                        cn = min(4, nq - off)
                        cw = cn * 128
                        qcol = slice((j + off) * 128, (j + off) * 128 + cw)
                        strip = strip_psp.tile([128, 2, 512], F32, tag="strip",
                                               name="strip")
                        nc.tensor.matmul(strip[:, 0, 0:cw], kT2[0:D, jcol],
                                         qTp[0:D, qcol], start=True, stop=True)
                        nc.tensor.matmul(strip[:, 1, 0:cw], kT2[D:128, jcol],
                                         qTp[D:128, qcol], start=True, stop=True)
                        nc.scalar.activation(
                            exp_t[:, :, off * 128:off * 128 + cw],
                            strip[:, :, 0:cw], AF.Exp, bias=zero_b, scale=scale)
                        off += cn
                    nc.vector.tensor_mul(exp_t[:, :, 0:128], exp_t[:, :, 0:128],
                                         maskd_sb)
                    if nq == WB + 1:
                        nc.vector.tensor_mul(exp_t[:, :, WB * 128:(WB + 1) * 128],
                                             exp_t[:, :, WB * 128:(WB + 1) * 128],
                                             maskf_sb)

                def mm2pair(m):
                    q0, q1 = 2 * m, 2 * m + 1
                    acc = acc_psp.tile([D + 1, 2, 256], F32, tag="acc", name="acc")
                    first = True
                    if q0 - WB >= 0:
                        jj = q0 - WB
                        nc.tensor.matmul(
                            acc[:, :, 0:128], vext[:, jj, :],
                            exps[jj][:, :, (q0 - jj) * 128:(q0 - jj) * 128 + 128],
                            start=True, stop=False, skip_group_check=True)
                        first = False
                    for jj in range(max(0, q1 - WB), q0 + 1):
                        off = (q0 - jj) * 128
                        nc.tensor.matmul(
                            acc, vext[:, jj, :],
                            exps[jj][:, :, off:off + 256],
                            start=first, stop=False, skip_group_check=True)
                        first = False
                    nc.tensor.matmul(
                        acc[:, :, 128:256], vext[:, q1, :],
                        exps[q1][:, :, 0:128],
                        start=False, stop=True, skip_group_check=True)
                    eps_st[m] = dict(acc=acc)

                def epi1(m):
                    s_ = eps_st[m]
                    acc = s_["acc"]
                    rec = ep.tile([1, 2, 256], F32R, tag="rec", name="rec")
                    with nc.allow_low_precision(reason="f32r softmax denom recip"):
                        nc.vector.reciprocal(rec, acc[D:D + 1, :, :])
                    bc_ps = bc_psp.tile([D, 2, 256], F32, tag="bc", name="bc_ps")
                    nc.tensor.matmul(bc_ps, ones_r[:],
                                     rec[:].rearrange("p a b -> p (a b)"),
                                     start=True, stop=True)
                    s_["bc_ps"] = bc_ps

                def epi2(m):
                    s_ = eps_st.pop(m)
                    acc, bc_ps = s_["acc"], s_["bc_ps"]
                    scol = slice(2 * m * 128, (2 * m + 2) * 128)
                    bc_sb = ep.tile([D, 2, 256], F32, tag="bcs", name="bc_sb")
                    nc.vector.tensor_copy(bc_sb, bc_ps)
                    nc.vector.tensor_mul(aoTp[0:D, scol], acc[0:D, 0, :],
                                         bc_sb[:, 0, :])
                    qi = m // 2
                    if m % 2 == 0:
                        quad[qi] = ep.tile([D, 4, 128], F32R, tag="stg",
                                           name="stg_quad")
                    qv = quad[qi][:].rearrange("p a b -> p (a b)")
                    nc.vector.tensor_mul(qv[:, (m % 2) * 256:(m % 2) * 256 + 256],
                                         acc[0:D, 1, :], bc_sb[:, 1, :])
                    if m % 2 == 1:
                        qcols = slice(qi * 512, (qi + 1) * 512)
                        nc.sync.dma_start(aoTp[D:128, qcols], quad.pop(qi))
                        if ps == 1 and 3 in phases:
                            mm3_pend.extend(range(qi * 4, qi * 4 + 4))

                def mm3(sb):
                    scol = slice(sb * 128, (sb + 1) * 128)
                    y_sb = yp.tile([128, E], F32, tag="ysb", name="y_sb")
                    for nch in range(2):
                        ncol = slice(nch * 512, (nch + 1) * 512)
                        y_ps = y_psp.tile([128, 512], F32, tag="y", name="y_ps")
                        for i in range(2):
                            nc.tensor.matmul(y_ps, aoT[i][:, scol],
                                             wo_sb[i][:, ncol],
                                             start=(i == 0), stop=(i == 1),
                                             skip_group_check=True)
                        nc.vector.tensor_copy(y_sb[:, ncol], y_ps)
                    nc.sync.dma_start(y[sb, :, :], y_sb)

                for j in range(JB + 2):
                    if j < JB:
                        mm1_exp_masks(j)
                    if j % 2 == 1 and (j - 1) // 2 < JB // 2:
                        mm2pair((j - 1) // 2)
                    if j % 2 == 0 and j >= 2:
                        epi1(j // 2 - 1)
                    if j % 2 == 1 and j >= 3:
                        epi2((j - 3) // 2)
                    if mm3_pend:
                        mm3(mm3_pend.pop(0))
                while mm3_pend:
                    mm3(mm3_pend.pop(0))

    nc.compile()
    return nc


def _prep_core_inputs(c, x, ve, cos, sin, Wq, Wk, Wv, Wo, Wg):
    b = c // TP
    h0 = (c % TP) * HD
    xTc = np.ascontiguousarray(x[b].T)                       # [E, S]
    xtg = np.empty((NG, 8, 128, GW * 128), np.float32)
    for g in range(NG):
        for k in range(8):
            xtg[g, k] = xTc[k * 128:(k + 1) * 128,
                            g * GW * 128:(g + 1) * GW * 128]
    wqkv = np.concatenate([Wq[:, h0:h0 + HD], Wk, Wv], axis=1)  # [E, 384]
    wq_t = np.ascontiguousarray(wqkv.reshape(8, 128, QKW))
    wo = np.ascontiguousarray(Wo[h0:h0 + HD, :])
    ccss = np.concatenate([cos, cos, sin, -sin], axis=1)     # [S, 128]
    tgve = np.concatenate([ccss, 2.0 * ve[b]], axis=1)       # [S, 192]
    tgve = np.ascontiguousarray(tgve.reshape(SB, 128, 192).astype(np.float32))
    ii = np.arange(128)
    md = (ii[None, :] >= ii[:, None]).astype(np.float32)     # [ki, qi] qi>=ki
    mf = 1.0 - md
    maskd = np.tile(md, (1, 2)).reshape(128, 256)
    maskf = np.tile(mf, (1, 2)).reshape(128, 256)
    wg2 = np.concatenate([Wg, Wg], axis=1).astype(np.float32)
    return dict(xTg=xtg, wq_t=wq_t, wg=wg2, wo=wo, tgve=tgve,
                maskd=maskd, maskf=maskf)


def kernel(x, ve, cos, sin, Wq, Wk, Wv, Wo, Wg, window_size):
    assert int(window_size) == WIN
    x = np.asarray(x, np.float32)
    ve = np.asarray(ve, np.float32)
    cos = np.asarray(cos, np.float32)
    sin = np.asarray(sin, np.float32)
    Wq = np.asarray(Wq, np.float32)
    Wk = np.asarray(Wk, np.float32)
    Wv = np.asarray(Wv, np.float32)
    Wo = np.asarray(Wo, np.float32)
    Wg = np.asarray(Wg, np.float32)

    if _NC_CACHE[0] is None:
        _NC_CACHE[0] = _build()
    nc = _NC_CACHE[0]

    in_maps = [_prep_core_inputs(c, x, ve, cos, sin, Wq, Wk, Wv, Wo, Wg)
               for c in range(NCORES)]
    res = run_bass_kernel_spmd(nc, in_maps, core_ids=list(range(NCORES)),
                               trace=TRACE)
    LAST_RESULT[0] = res

    out = np.zeros((B, S, E), np.float32)
    for c in range(NCORES):
        out[c // TP] += res.results[c]["y"].reshape(S, E)
    return out
